# revision 76
# baseline (speedup 1.0000x reference)
"""Trainium2 Bass kernel for nn_CrossAttention (B=4, L=2048, D=1024, H=8).

Sharding: 8 cores = 4 batches x 2 query-halves (data parallel over B x Lq).

Host-side prep (per core):
  Qn = LN(Q_slice)*pre_g + pre_b ; Kn = LN(K_b)*pre_g + pre_b  (numpy fp32)
  QT8/KT8 = fp8e4m3(Qn^T / Kn^T) in DoubleRow pair layout
  WQ8/WK8 = fp8e4m3(Wq*sq / Wk*sk) in DoubleRow pair layout (sq/sk pow2)
  VT = bf16(V_b^T), WV/WO = bf16
All feature-major operands arrive pre-transposed from the host, so the
device does zero input transposes.

Device (per core):
  k^T/q^T projections: fp8e4m3 DoubleRow matmuls (2 contraction chunks per
    matmul, 2x PE rate), interleaved per-head into the attention stream
  v = V^T-stationary @ Wv (bf16), interleaved into head 0's stream
  flat (head, key-tile) attention stream with a depth-2 pending pipeline
  crossing head boundaries:
     S^T[128k, 1024q] = kth-blk stationary @ qth moving  (bf16, psum)
     ex = exp(S^T * 1/(TEMP*sq*sk))  (ACT, one instr per key tile)
     sums[q]: ex-chunk-as-stationary @ ones 1-row matmuls into a small psum
              tile, accumulated into SBUF by DVE
     O^T[hd, q] += vnat-blk stationary @ ex  (bf16)
     PE latency gaps are filled with next head's projection J-blocks and
     previous head's O^T transposes
  D: transpose O^T back per head scaled by 1/sums, LN (query-major),
     transpose to feature-major, G = O_ln @ Wo (stationary=o_lnT blocks),
     out = O_ln + gelu(G).
"""

import numpy as np
import ml_dtypes

P = 128
D = 1024
H = 8
HD = 128
LQ = 1024  # per-core query rows
LK = 2048
N_CORES = 8
TEMP = 32.0  # sqrt(D)
EPS = 1e-5
NPAIR = 4  # D // (2*P) contraction chunk-pairs for fp8 DoubleRow
NCH = 8    # D // P contraction chunks
NJC = LK // P   # 16 key tiles
NQT = LQ // P   # 8 query tiles

QK_FP8 = True
INTERLEAVE = True

_PROGRAM_CACHE = {}

BF16 = ml_dtypes.bfloat16
FP8 = ml_dtypes.float8_e4m3


def _build_program(
    qk_fp8: bool,
    trivial_ln: bool,
    exp_scale: float,
    use_gelu: bool = True,
    trunc: int | None = None,
):
    import concourse.bacc as bacc
    import concourse.mybir as mybir
    import concourse.tile as tile
    from contextlib import ExitStack

    F32 = mybir.dt.float32
    BF = mybir.dt.bfloat16
    F8 = mybir.dt.float8e4
    AF = mybir.ActivationFunctionType
    SUB = mybir.AluOpType.subtract
    MULT = mybir.AluOpType.mult
    DR = mybir.MatmulPerfMode.DoubleRow

    nc = bacc.Bacc("TRN2", target_bir_lowering=False, debug=False)

    # ---- DRAM I/O ----
    if qk_fp8:
        QT_d = nc.dram_tensor("QT", [NPAIR * P, 2 * LQ], F8, kind="ExternalInput")
        KT_d = nc.dram_tensor("KT", [NPAIR * P, 2 * LK], F8, kind="ExternalInput")
        WQ_d = nc.dram_tensor("WQ", [NPAIR * P, 2 * D], F8, kind="ExternalInput")
        WK_d = nc.dram_tensor("WK", [NPAIR * P, 2 * D], F8, kind="ExternalInput")
    else:
        QT_d = nc.dram_tensor("QT", [D, LQ], BF, kind="ExternalInput")
        KT_d = nc.dram_tensor("KT", [D, LK], BF, kind="ExternalInput")
        WQ_d = nc.dram_tensor("WQ", [D, D], BF, kind="ExternalInput")
        WK_d = nc.dram_tensor("WK", [D, D], BF, kind="ExternalInput")
    # VT is in jc-major packed layout: VT[jc*P + r, c*P + k] = V[jc*P + k, c*P + r]
    VT_d = nc.dram_tensor("VT", [LK, D], BF, kind="ExternalInput")
    WV_d = nc.dram_tensor("WV", [D, D], BF, kind="ExternalInput")
    WO_d = nc.dram_tensor("WO", [D, D], BF, kind="ExternalInput")
    ID_d = nc.dram_tensor("IDENT", [P, P], BF, kind="ExternalInput")
    ONES_d = nc.dram_tensor("ONES", [P, 1], BF, kind="ExternalInput")
    if not trivial_ln:
        LNG_d = nc.dram_tensor("LNG_B", [P, D], F32, kind="ExternalInput")
        LNB_d = nc.dram_tensor("LNB_B", [P, D], F32, kind="ExternalInput")
    OUT = nc.dram_tensor("OUT", [LQ, D], F32, kind="ExternalOutput")

    with tile.TileContext(nc) as tc, ExitStack() as top:
        singles = top.enter_context(tc.tile_pool(name="singles", bufs=1))
        ident = singles.tile([P, P], BF)
        ones = singles.tile([P, 1], BF)
        eps_t = singles.tile([P, 1], F32)
        nc.vector.memset(eps_t[:], EPS)
        if not trivial_ln:
            lng = singles.tile([P, D], F32)
            lnb = singles.tile([P, D], F32)
            nc.sync.dma_start(lng[:], LNG_d.ap())
            nc.sync.dma_start(lnb[:], LNB_d.ap())

        # ---- persistent activations ----
        act_pool = top.enter_context(tc.tile_pool(name="acts", bufs=1))
        kth = [act_pool.tile([P, LK], BF, name=f"kth{h}") for h in range(H)]
        qth = [act_pool.tile([P, LQ], BF, name=f"qth{h}") for h in range(H)]
        vnat = [act_pool.tile([P, D], BF, name=f"vnat{t}") for t in range(NJC)]
        o_nat = [act_pool.tile([P, D], BF, name=f"onat{t}") for t in range(NQT)]
        oev_pool = top.enter_context(tc.tile_pool(name="oev", bufs=3))
        recipN = singles.tile([P, H * NQT], F32)

        # ---- psum pools (8 banks total: st 2x2 + ot 2 + small 2) ----
        st_ps = top.enter_context(tc.tile_pool(name="st_ps", bufs=2, space="PSUM"))
        ex_pool = top.enter_context(tc.tile_pool(name="ex", bufs=6))
        ot_ps_pool = top.enter_context(tc.tile_pool(name="ot_ps", bufs=1, space="PSUM"))
        small_ps = top.enter_context(tc.tile_pool(name="small", bufs=2, space="PSUM"))

        # softmax sums accumulate in SBUF (cols h*8+t), fed by per-jc psum tiles
        sums_sb = singles.tile([P, H * NQT], F32)
        nc.vector.memset(sums_sb[:], 0.0)

        # ---- weight / input loads (phased pools, closed when done; must be
        # created after all persistent pools to keep LIFO release order) ----
        kq_store = ExitStack()
        wpool = kq_store.enter_context(tc.tile_pool(name="wpool", bufs=1))
        xpool = kq_store.enter_context(tc.tile_pool(name="xpool", bufs=1))
        v_store = ExitStack()
        wvp = v_store.enter_context(tc.tile_pool(name="wvp", bufs=1))
        vtp = v_store.enter_context(tc.tile_pool(name="vtp", bufs=3))

        if qk_fp8:
            wq8 = [wpool.tile([P, 2 * D], F8, name=f"wq8_{p}") for p in range(NPAIR)]
            wk8 = [wpool.tile([P, 2 * D], F8, name=f"wk8_{p}") for p in range(NPAIR)]
            qt8 = [xpool.tile([P, 2 * LQ], F8, name=f"qt8_{p}") for p in range(NPAIR)]
            kt8 = [xpool.tile([P, 2 * LK], F8, name=f"kt8_{p}") for p in range(NPAIR)]

            for p in range(NPAIR):
                enga = nc.sync if p % 2 == 0 else nc.scalar
                engb = nc.scalar if p % 2 == 0 else nc.sync
                enga.dma_start(wk8[p][:], WK_d.ap()[p * P : (p + 1) * P, :])
                engb.dma_start(kt8[p][:], KT_d.ap()[p * P : (p + 1) * P, :])
            for p in range(NPAIR):
                eng = nc.sync if p % 2 == 0 else nc.scalar
                eng.dma_start(wq8[p][:], WQ_d.ap()[p * P : (p + 1) * P, :])
                eng.dma_start(qt8[p][:], QT_d.ap()[p * P : (p + 1) * P, :])
        else:
            wqb = [wpool.tile([P, D], BF, name=f"wqb_{c}") for c in range(NCH)]
            wkb = [wpool.tile([P, D], BF, name=f"wkb_{c}") for c in range(NCH)]
            qtb = [xpool.tile([P, LQ], BF, name=f"qtb_{c}") for c in range(NCH)]
            ktb = [xpool.tile([P, LK], BF, name=f"ktb_{c}") for c in range(NCH)]
            for c in range(NCH):
                nc.sync.dma_start(wkb[c][:], WK_d.ap()[c * P : (c + 1) * P, :])
                nc.sync.dma_start(ktb[c][:], KT_d.ap()[c * P : (c + 1) * P, :])
            for c in range(NCH):
                nc.sync.dma_start(wqb[c][:], WQ_d.ap()[c * P : (c + 1) * P, :])
                nc.sync.dma_start(qtb[c][:], QT_d.ap()[c * P : (c + 1) * P, :])

        # ident/ones are first needed ~15us in (sums / D1) - keep them off
        # the critical head of the DMA queues
        nc.sync.dma_start(ones[:], ONES_d.ap())
        nc.sync.dma_start(ident[:], ID_d.ap())
        wv = [wvp.tile([P, D], BF, name=f"wv_{c}") for c in range(NCH)]
        for c in range(NCH):
            nc.scalar.dma_start(wv[c][:], WV_d.ap()[c * P : (c + 1) * P, :])

        def _kq_block(h, tiles, w8, x8, J):
            """Emit one 512-col projection J-block for head h."""
            ps = small_ps.tile([P, 512], F32, tag="sm")
            if qk_fp8:
                for p in range(NPAIR):
                    wap = w8[p][:].rearrange("r (i c) -> r i c", i=2)
                    xap = x8[p][:].rearrange("r (i c) -> r i c", i=2)
                    nc.tensor.matmul(
                        ps[:],
                        wap[:, :, h * HD : (h + 1) * HD],
                        xap[:, :, J * 512 : (J + 1) * 512],
                        start=(p == 0),
                        stop=(p == NPAIR - 1),
                        perf_mode=DR,
                    )
            else:
                for c in range(NCH):
                    nc.tensor.matmul(
                        ps[:],
                        w8[c][:, h * HD : (h + 1) * HD],
                        x8[c][:, J * 512 : (J + 1) * 512],
                        start=(c == 0),
                        stop=(c == NCH - 1),
                    )
            nc.vector.tensor_copy(tiles[h][:, J * 512 : (J + 1) * 512], ps[:])

        def kq_proj_blocks(h):
            """Closures emitting each projection J-block for head h (6 total)."""
            if qk_fp8:
                specs = ((kth, wk8, kt8, LK), (qth, wq8, qt8, LQ))
            else:
                specs = ((kth, wkb, ktb, LK), (qth, wqb, qtb, LQ))
            blocks = []
            for tiles, w, x, L in specs:
                for J in range(L // 512):
                    blocks.append(
                        lambda h=h, tiles=tiles, w=w, x=x, J=J: _kq_block(
                            h, tiles, w, x, J
                        )
                    )
            return blocks

        def kq_proj(h):
            for blk in kq_proj_blocks(h):
                blk()

        vt_tiles = {}

        def vt_prefetch(jc):
            if jc < NJC and jc not in vt_tiles:
                vt = vtp.tile([P, D], BF, tag="vt")
                nc.sync.dma_start(vt[:], VT_d.ap()[jc * P : (jc + 1) * P, :])
                vt_tiles[jc] = vt

        def v_proj_half(jc, s):
            """vnat[jc][:, s-half] = (V^T blocks)^T @ Wv, natural layout."""
            vt_prefetch(jc)
            vt = vt_tiles[jc]
            ps = small_ps.tile([P, 512], F32, tag="sm")
            for c in range(NCH):
                nc.tensor.matmul(
                    ps[:],
                    vt[:, c * P : (c + 1) * P],
                    wv[c][:, s * 512 : (s + 1) * 512],
                    start=(c == 0),
                    stop=(c == NCH - 1),
                )
            nc.vector.tensor_copy(vnat[jc][:, s * 512 : (s + 1) * 512], ps[:])
            if s == 1:
                del vt_tiles[jc]

        def v_proj(jc):
            v_proj_half(jc, 0)
            v_proj_half(jc, 1)

        def d1_start(h, ot_ps):
            """Evict O^T(h) from psum and compute 1/sums for head h."""
            oev = oev_pool.tile([P, LQ], BF, tag="oev", name=f"oev{h % 2}")
            nc.vector.tensor_copy(oev[:], ot_ps[:])
            nc.vector.reciprocal(
                recipN[:, h * NQT : (h + 1) * NQT],
                sums_sb[:, h * NQT : (h + 1) * NQT],
            )
            return oev

        def d1_tp(h, oev, t):
            """Transpose one O^T(h) block to query-major, scaled by 1/sums."""
            tp = small_ps.tile([P, P], BF, tag="sm")
            nc.tensor.transpose(tp[:], oev[:, t * P : (t + 1) * P], ident[:])
            nc.vector.tensor_scalar_mul(
                out=o_nat[t][:, h * HD : (h + 1) * HD],
                in0=tp[:],
                scalar1=recipN[:, h * NQT + t : h * NQT + t + 1],
            )

        def d1_head(h, ot_ps):
            oev = d1_start(h, ot_ps)
            for t in range(NQT):
                d1_tp(h, oev, t)

        # ================= main attention loop =================
        n_heads = H if (trunc is None or trunc <= 0 or trunc >= 10) else trunc

        def sums_mms(h, jc, ex):
            sp = small_ps.tile([P, 512], F32, tag="sm")
            for qc in range(NQT):
                nc.tensor.matmul(
                    sp[:, qc : qc + 1],
                    ex[:, qc * P : (qc + 1) * P],
                    ones[:],
                    start=(qc == 0),
                    stop=(qc == NQT - 1),
                    skip_group_check=True,
                )
            nc.vector.tensor_add(
                out=sums_sb[:, h * NQT : (h + 1) * NQT],
                in0=sums_sb[:, h * NQT : (h + 1) * NQT],
                in1=sp[:, 0:NQT],
            )

        kq_proj(0)
        interleave = INTERLEAVE and (trunc is None or trunc >= 10)
        run_heads = n_heads if trunc != 0 else 0

        # Flat (h, jc) stream with a depth-2 pending pipeline that crosses
        # head boundaries, so the PE never drains between heads.
        pend = []
        fillers = []
        ot_of = {}

        def drain_one():
            ph, pjc, pex = pend.pop(0)
            if pjc == 0:
                ot_of[ph] = ot_ps_pool.tile([P, LQ], F32, tag="ot", name=f"ot{ph}")
            sums_mms(ph, pjc, pex)
            for s in range(2):
                nc.tensor.matmul(
                    ot_of[ph][:, s * 512 : (s + 1) * 512],
                    vnat[pjc][:, ph * HD : (ph + 1) * HD],
                    pex[:, s * 512 : (s + 1) * 512],
                    start=(pjc == 0),
                    stop=(pjc == NJC - 1),
                )
            if pjc == NJC - 1:
                ot = ot_of.pop(ph)
                oev = d1_start(ph, ot)
                fillers.extend(
                    ("d1", lambda t=t, oev=oev, hh=ph: d1_tp(hh, oev, t))
                    for t in range(NQT)
                )

        for h in range(run_heads):
            if h == 1:
                v_store.close()
            if h > 0 and not interleave:
                kq_proj(h)
            if h == H - 1:
                kq_store.close()
                # load Wo into the space freed by the q/k inputs
                wo_pool = top.enter_context(tc.tile_pool(name="wo", bufs=1))
                wo = [wo_pool.tile([P, D], BF, name=f"wo_{c}") for c in range(NCH)]
                for c in range(NCH):
                    nc.sync.dma_start(wo[c][:], WO_d.ap()[c * P : (c + 1) * P, :])
            if interleave and h + 1 < n_heads:
                fillers.extend(("proj", b) for b in kq_proj_blocks(h + 1))
            for jc in range(NJC):
                st = st_ps.tile([P, LQ], F32, tag="st")
                for s in range(2):
                    nc.tensor.matmul(
                        st[:, s * 512 : (s + 1) * 512],
                        kth[h][:, jc * P : (jc + 1) * P],
                        qth[h][:, s * 512 : (s + 1) * 512],
                        start=True,
                        stop=True,
                    )
                ex = ex_pool.tile([P, LQ], BF, tag="ex")
                nc.scalar.activation(ex[:], st[:], AF.Exp, scale=exp_scale)
                if h == 0:
                    if jc == 0:
                        vt_prefetch(0)
                        vt_prefetch(1)
                    vt_prefetch(jc + 2)
                    v_proj_half(jc, 0)
                if fillers and not (
                    fillers[0][0] == "d1"
                    and 0 < h < H - 1
                    and jc % 8 == 5
                ):
                    fillers.pop(0)[1]()
                pend.append((h, jc, ex))
                if len(pend) > 3:
                    drain_one()
                if h == 0:
                    v_proj_half(jc, 1)
        while pend:
            drain_one()
        for _, f in fillers:
            f()
        if trunc is not None and trunc < 10:
            # truncated build: dump partial state and stop
            if trunc == 0:
                for h in range(1, H):
                    kq_proj(h)
                for jc in range(NJC):
                    v_proj(jc)
                src_tiles = qth
            elif trunc < 10:
                d1_head(n_heads - 1, prev_ot)
                src_tiles = o_nat
            v_store.close()
            kq_store.close()
            dump_pool = top.enter_context(tc.tile_pool(name="dump", bufs=2))
            for t in range(NQT):
                tmp = dump_pool.tile([P, D], F32, tag="dump")
                nc.vector.tensor_copy(tmp[:], src_tiles[t][:])
                nc.sync.dma_start(OUT.ap()[t * P : (t + 1) * P, :], tmp[:])

        if trunc is None:
            # ========== D: per-tile LN -> transpose -> Wo -> gelu+res ==========
            fin = top.enter_context(tc.tile_pool(name="fin", bufs=3))
            oln_pool = top.enter_context(tc.tile_pool(name="oln", bufs=1))
            olnT_pool = top.enter_context(tc.tile_pool(name="olnT", bufs=1))
            o_ln = [oln_pool.tile([P, D], BF, name=f"oln{t}") for t in range(NQT)]
            o_lnT = [olnT_pool.tile([P, LQ], BF, name=f"olnT{c}") for c in range(NCH)]

            def layernorm(pool, x_tile, out_tile):
                xr = x_tile[:].rearrange("p (n f) -> p n f", f=512)
                stats = pool.tile([P, 2, nc.vector.BN_STATS_DIM], F32, tag="ln_stats")
                for i in range(2):
                    nc.vector.bn_stats(out=stats[:, i, :], in_=xr[:, i, :])
                mv = pool.tile([P, nc.vector.BN_AGGR_DIM], F32, tag="ln_mv")
                nc.vector.bn_aggr(out=mv[:], in_=stats[:])
                rstd = pool.tile([P, 1], F32, tag="ln_rstd")
                nc.scalar.activation(
                    out=rstd[:], in_=mv[:, 1:2], func=AF.Sqrt, bias=eps_t[:]
                )
                nc.vector.reciprocal(out=rstd[:], in_=rstd[:])
                nc.vector.tensor_scalar(
                    out=out_tile[:],
                    in0=x_tile[:],
                    scalar1=mv[:, 0:1],
                    scalar2=rstd[:],
                    op0=SUB,
                    op1=MULT,
                )

            def ln_tile(t):
                layernorm(fin, o_nat[t], o_ln[t])
                if not trivial_ln:
                    nc.vector.tensor_mul(out=o_ln[t][:], in0=o_ln[t][:], in1=lng[:])
                    nc.vector.tensor_add(out=o_ln[t][:], in0=o_ln[t][:], in1=lnb[:])

            ln_tile(0)
            for t in range(NQT):
                if t + 1 < NQT:
                    ln_tile(t + 1)
                for c in range(NCH):
                    tp = small_ps.tile([P, P], BF, tag="sm")
                    nc.tensor.transpose(
                        tp[:], o_ln[t][:, c * P : (c + 1) * P], ident[:]
                    )
                    nc.vector.tensor_copy(o_lnT[c][:, t * P : (t + 1) * P], tp[:])
            for t in range(NQT):
                for s in range(2):
                    sl = slice(s * 512, (s + 1) * 512)
                    g = small_ps.tile([P, 512], F32, tag="sm")
                    for c in range(NCH):
                        nc.tensor.matmul(
                            g[:],
                            o_lnT[c][:, t * P : (t + 1) * P],
                            wo[c][:, sl],
                            start=(c == 0),
                            stop=(c == NCH - 1),
                        )
                    gel = fin.tile([P, 512], F32, tag="gelu")
                    nc.scalar.activation(
                        gel[:], g[:], AF.Gelu if use_gelu else AF.Sigmoid
                    )
                    outt = fin.tile([P, 512], F32, tag="outsb")
                    nc.vector.tensor_add(
                        out=outt[:], in0=gel[:], in1=o_ln[t][:, sl]
                    )
                    oeng = nc.sync if (2 * t + s) % 2 == 0 else nc.scalar
                    oeng.dma_start(OUT.ap()[t * P : (t + 1) * P, sl], outt[:])

    nc.compile()
    return nc


def _pair_layout(xT: np.ndarray) -> np.ndarray:
    """[D, L] -> DoubleRow pair layout [NPAIR*P, 2*L]:
    row block p, col block i*L+q  <-  xT[(2p+i)*P + r, q]."""
    Dd, L = xT.shape
    return (
        xT.reshape(NPAIR, 2, P, L).transpose(0, 2, 1, 3).reshape(NPAIR * P, 2 * L)
    )


def _host_ln(x: np.ndarray, g: np.ndarray, b: np.ndarray) -> np.ndarray:
    mu = x.mean(axis=-1, keepdims=True, dtype=np.float32)
    xc = x - mu
    var = np.mean(xc * xc, axis=-1, keepdims=True, dtype=np.float32)
    return xc * (1.0 / np.sqrt(var + EPS)) * g + b


def _pow2_scale(w: np.ndarray) -> float:
    """Power-of-two scale mapping w's std into ~O(1) for fp8e4m3."""
    s = float(np.std(w))
    if s <= 0 or not np.isfinite(s):
        return 1.0
    return float(2.0 ** np.floor(np.log2(0.5 / s)))


def kernel(Q, K, V, Wq, Wk, Wv, Wo, pre_g, pre_b, ln_g, ln_b):
    from concourse.bass_utils import run_bass_kernel_spmd

    Q = np.asarray(Q, np.float32)
    K = np.asarray(K, np.float32)
    V = np.asarray(V, np.float32)
    Wq = np.asarray(Wq, np.float32)
    Wk = np.asarray(Wk, np.float32)
    Wv = np.asarray(Wv, np.float32)
    Wo = np.asarray(Wo, np.float32)
    pre_g = np.asarray(pre_g, np.float32)
    pre_b = np.asarray(pre_b, np.float32)
    ln_g = np.asarray(ln_g, np.float32)
    ln_b = np.asarray(ln_b, np.float32)

    trivial_ln = bool(np.all(ln_g == 1.0) and np.all(ln_b == 0.0))

    B = Q.shape[0]
    Qn = _host_ln(Q, pre_g, pre_b)
    Kn = _host_ln(K, pre_g, pre_b)

    if QK_FP8:
        sq = _pow2_scale(Wq)
        sk = _pow2_scale(Wk)
        WQ8 = _pair_layout(np.ascontiguousarray(Wq * sq)).astype(FP8)
        WK8 = _pair_layout(np.ascontiguousarray(Wk * sk)).astype(FP8)
        exp_scale = 1.0 / (TEMP * sq * sk)
    else:
        WQb = np.ascontiguousarray(Wq).astype(BF16)
        WKb = np.ascontiguousarray(Wk).astype(BF16)
        exp_scale = 1.0 / TEMP

    key = (QK_FP8, trivial_ln, exp_scale)
    if key not in _PROGRAM_CACHE:
        _PROGRAM_CACHE[key] = _build_program(*key)
    nc = _PROGRAM_CACHE[key]

    WVb = np.ascontiguousarray(Wv).astype(BF16)
    WOb = np.ascontiguousarray(Wo).astype(BF16)
    ident = np.eye(P, dtype=np.float32).astype(BF16)
    ones = np.ones((P, 1), np.float32).astype(BF16)

    in_maps = []
    for core in range(N_CORES):
        b, half = core // 2, core % 2
        QnT = np.ascontiguousarray(Qn[b, half * LQ : (half + 1) * LQ, :].T)
        KnT = np.ascontiguousarray(Kn[b].T)
        # jc-major packed V^T: VT_jc[jc*P + r, c*P + k] = V[b][jc*P + k, c*P + r]
        VTjc = (
            V[b]
            .reshape(NJC, P, NCH, P)
            .transpose(0, 3, 2, 1)
            .reshape(LK, D)
        )
        m = {
            "VT": np.ascontiguousarray(VTjc).astype(BF16),
            "WV": WVb,
            "WO": WOb,
            "IDENT": ident,
            "ONES": ones,
        }
        if QK_FP8:
            m["QT"] = _pair_layout(QnT).astype(FP8)
            m["KT"] = _pair_layout(KnT).astype(FP8)
            m["WQ"] = WQ8
            m["WK"] = WK8
        else:
            m["QT"] = QnT.astype(BF16)
            m["KT"] = KnT.astype(BF16)
            m["WQ"] = WQb
            m["WK"] = WKb
        if not trivial_ln:
            m["LNG_B"] = np.tile(ln_g[None, :], (P, 1))
            m["LNB_B"] = np.tile(ln_b[None, :], (P, 1))
        in_maps.append(m)

    res = run_bass_kernel_spmd(nc, in_maps, core_ids=list(range(N_CORES)))
    out = np.empty((B, 2 * LQ, D), np.float32)
    for core in range(N_CORES):
        b, half = core // 2, core % 2
        out[b, half * LQ : (half + 1) * LQ, :] = res.results[core]["OUT"]
    return out
